# revision 11
# baseline (speedup 1.0000x reference)
"""Trainium2 Bass kernel for nn_Decoder (mask-multiply + Linear(512->16) + overlap-add).

Full-input contract: kernel(mixture_w, est_mask, W) -> [4, 128008] float32.

Sharding: 8 cores = 4 batches x 2 K-halves (8000 frames each).

Raw-bass (explicit semaphores) pipeline per core, chunk = 500 frames (16 chunks):
  SP  : one 2MB DMA per chunk loads stacked [mw; em] slice into x_buf[b]
  DVE : est[b] = x[:,0] * x[:,1]  (float32r out), and the overlap-add
        res[:,k] = psA[:,k] + sbB[:,k-1]
  PE  : 8 matmuls (W.T halves stationary, est moving, float32r full rate)
        -> psA[8,500] (j=0..8), psB[8,500] (j=8..16); then 4 transposes of
        res into k-major pst[125,32] (software-pipelined one chunk behind)
  ACT : evacuates psB->sbB and pst->ct (ScalarE is PSUM-fast), issues the
        16KB output DMA per chunk on its own HWDGE ring
Host adds the 8-sample seam between the two K-halves of each batch.

Every instruction carries at most one semaphore wait (ISA limit); extra
dependencies are expressed as standalone wait_ge instructions.
"""

import numpy as np

import concourse.bass as bass
import concourse.mybir as mybir
from concourse.bass_utils import run_bass_kernel_spmd

F32 = mybir.dt.float32
F32R = mybir.dt.float32r

B, N, K, L = 4, 512, 16000, 16
STEP = L // 2              # 8
KLOC = K // 2              # 8000 frames per core
TLOC = STEP * (KLOC - 1) + L   # 64008 local output samples
CHUNK = 500                # frames per chunk (<=512 psum bank)
NSTEPS = KLOC // CHUNK     # 16


def build_nc(reps: int = 1) -> bass.Bass:
    G = NSTEPS * reps  # global chunk count (reps>1 = bench-only steady-state loop)
    nc = bass.Bass()
    x = nc.dram_tensor("x", [2, N, KLOC], F32, kind="ExternalInput")
    wt = nc.dram_tensor("wt", [N, L], F32, kind="ExternalInput")
    ident = nc.dram_tensor("ident", [8, 8], F32, kind="ExternalInput")
    out = nc.dram_tensor("out", [TLOC], F32, kind="ExternalOutput")

    x_r = x.rearrange("t (ni p) k -> p t ni k", p=128)
    wt_r = wt.rearrange("(ni p) l -> p ni l", p=128)

    from contextlib import ExitStack

    with ExitStack() as stk:
        xb0 = stk.enter_context(nc.sbuf_tensor([128, 2, 4, CHUNK], F32))
        xb1 = stk.enter_context(nc.sbuf_tensor([128, 2, 4, CHUNK], F32))
        eb0 = stk.enter_context(nc.sbuf_tensor([128, 4, CHUNK], F32R))
        eb1 = stk.enter_context(nc.sbuf_tensor([128, 4, CHUNK], F32R))
        wt_f = stk.enter_context(nc.sbuf_tensor([128, 4, L], F32))
        wt_sb = stk.enter_context(nc.sbuf_tensor([128, 4, L], F32R))
        id_sb = stk.enter_context(nc.sbuf_tensor([8, 8], F32))
        sbB0 = stk.enter_context(nc.sbuf_tensor([8, CHUNK], F32))
        sbB1 = stk.enter_context(nc.sbuf_tensor([8, CHUNK], F32))
        res0 = stk.enter_context(nc.sbuf_tensor([8, CHUNK], F32))
        res1 = stk.enter_context(nc.sbuf_tensor([8, CHUNK], F32))
        ct0 = stk.enter_context(nc.sbuf_tensor([125, 32], F32))
        ct1 = stk.enter_context(nc.sbuf_tensor([125, 32], F32))
        ct_tail = stk.enter_context(nc.sbuf_tensor([1, 8], F32))
        psA0 = stk.enter_context(nc.psum_tensor([8, CHUNK], F32))
        psA1 = stk.enter_context(nc.psum_tensor([8, CHUNK], F32))
        psB0 = stk.enter_context(nc.psum_tensor([8, CHUNK], F32))
        psB1 = stk.enter_context(nc.psum_tensor([8, CHUNK], F32))
        pst0 = stk.enter_context(nc.psum_tensor([125, 32], F32))
        pst1 = stk.enter_context(nc.psum_tensor([125, 32], F32))
        pstail = stk.enter_context(nc.psum_tensor([1, 8], F32))
        wsem = stk.enter_context(nc.semaphore())   # wt+ident DMAs, +16 each
        dsem0 = stk.enter_context(nc.semaphore())  # even-chunk x DMAs, +16
        dsem1 = stk.enter_context(nc.semaphore())  # odd-chunk x DMAs, +16
        msem = stk.enter_context(nc.semaphore())   # DVE: wt cast copy + mults
        asem = stk.enter_context(nc.semaphore())   # DVE: overlap-add per chunk
        psem_a = stk.enter_context(nc.semaphore())  # PE: psA group per chunk
        psem_b = stk.enter_context(nc.semaphore())  # PE: psB group per chunk
        psem_t = stk.enter_context(nc.semaphore())  # PE: transposes per chunk
        esem = stk.enter_context(nc.semaphore())   # ACT: psB evac per chunk
        ctsem = stk.enter_context(nc.semaphore())  # ACT: ct copy per chunk
        osem0 = stk.enter_context(nc.semaphore())  # even-chunk out DMAs, +16
        osem1 = stk.enter_context(nc.semaphore())  # odd-chunk out DMAs, +16
        block = stk.enter_context(nc.Block())
        xb = [xb0, xb1]
        eb = [eb0, eb1]
        sbB = [sbB0, sbB1]
        res = [res0, res1]
        ct = [ct0, ct1]
        psA = [psA0, psA1]
        psB = [psB0, psB1]
        pst = [pst0, pst1]

        dsem = [dsem0, dsem1]
        osem = [osem0, osem1]

        @block.sync
        def _(sync):
            sync.dma_start(wt_f[:], wt_r).then_inc(wsem, 16)
            sync.dma_start(id_sb[:], ident[:]).then_inc(wsem, 16)
            for g in range(G):
                s, b = g % NSTEPS, g % 2
                if g >= 2:
                    # x_buf[b] last read by mult(g-2)
                    sync.wait_ge(msem, g)  # wt-copy(1) + mults up to g-2 -> g-1+1
                sync.dma_start(
                    xb[b][:], x_r[:, :, :, s * CHUNK : (s + 1) * CHUNK]
                ).then_inc(dsem[b], 16)

        @block.vector
        def _(vector):
            vector.wait_ge(wsem, 32)
            nc.vector.tensor_copy(out=wt_sb[:], in_=wt_f[:]).then_inc(msem, 1)

            def mult(g):
                b = g % 2
                vector.wait_ge(dsem[b], 16 * (g // 2 + 1))
                if g >= 2:
                    vector.wait_ge(psem_b, g - 1)  # est[b] read by MMs(g-2)
                nc.vector.tensor_mul(
                    out=eb[b][:], in0=xb[b][:, 0], in1=xb[b][:, 1]
                ).then_inc(msem, 1)

            mult(0)
            if G > 1:
                mult(1)
            for g in range(G):
                b = g % 2
                # overlap-add for chunk g
                vector.wait_ge(psem_a, g + 1)
                vector.wait_ge(esem, g + 1)
                if g >= 2:
                    vector.wait_ge(psem_t, g - 1)  # res[b] read by TR(g-2)
                nc.vector.tensor_add(
                    out=res[b][:, 1:CHUNK],
                    in0=psA[b][:, 1:CHUNK],
                    in1=sbB[b][:, 0 : CHUNK - 1],
                )
                if g == 0:
                    nc.vector.tensor_copy(
                        out=res[b][:, 0:1], in_=psA[b][:, 0:1]
                    ).then_inc(asem, 1)
                else:
                    nc.vector.tensor_add(
                        out=res[b][:, 0:1],
                        in0=psA[b][:, 0:1],
                        in1=sbB[1 - b][:, CHUNK - 1 : CHUNK],
                    ).then_inc(asem, 1)
                if g + 2 < G:
                    mult(g + 2)

        @block.tensor
        def _(tensor):
            def transpose_group(g):
                b = g % 2
                tensor.wait_ge(asem, g + 1)
                if g >= 2:
                    tensor.wait_ge(ctsem, g - 1)  # pst[b] read by ct-copy(g-2)
                for t in range(4):
                    mm = nc.tensor.transpose(
                        pst[b][:, 8 * t : 8 * t + 8], res[b][:, t::4], id_sb[:]
                    )
                    if t == 3:
                        mm.then_inc(psem_t, 1)

            tensor.wait_ge(wsem, 32)  # id_sb loaded (for transposes)
            tensor.wait_ge(msem, 2)  # wt_sb + est(0)
            for g in range(G):
                b = g % 2
                if g >= 1:
                    tensor.wait_ge(msem, g + 2)  # est(g) ready
                if g >= 2:
                    tensor.wait_ge(asem, g - 1)  # psA[b] read by add(g-2)
                for ni in range(4):
                    mm = nc.tensor.matmul(
                        psA[b][:],
                        wt_sb[:, ni, 0:STEP],
                        eb[b][:, ni],
                        start=(ni == 0),
                        stop=(ni == 3),
                    )
                    if ni == 3:
                        mm.then_inc(psem_a, 1)
                if g >= 2:
                    tensor.wait_ge(esem, g - 1)  # psB[b] read by evac(g-2)
                for ni in range(4):
                    mm = nc.tensor.matmul(
                        psB[b][:],
                        wt_sb[:, ni, STEP:L],
                        eb[b][:, ni],
                        start=(ni == 0),
                        stop=(ni == 3),
                    )
                    if ni == 3:
                        mm.then_inc(psem_b, 1)
                # transposes run one chunk behind so PE never waits on the
                # DVE/ACT round-trip of the current chunk
                if g >= 1:
                    transpose_group(g - 1)
            transpose_group(G - 1)
            # tail: transpose sbB[last][:, CHUNK-1] -> pstail [1, 8]
            tensor.wait_ge(esem, G)
            nc.tensor.transpose(
                pstail[:], sbB[(G - 1) % 2][:, CHUNK - 1 : CHUNK], id_sb[:]
            ).then_inc(psem_t, 1)

        @block.scalar
        def _(scalar):
            for g in range(G):
                s, b = g % NSTEPS, g % 2
                scalar.wait_ge(psem_b, g + 1)
                if g >= 1:
                    scalar.wait_ge(asem, g)  # sbB[b] read by add(g-1) boundary
                nc.scalar.copy(out=sbB[b][:], in_=psB[b][:]).then_inc(esem, 1)
                scalar.wait_ge(psem_t, g + 1)
                if g >= 2:
                    # ct[b] read by out-dma(g-2); g//2 same-parity DMAs issued
                    scalar.wait_ge(osem[b], 16 * (g // 2))
                nc.scalar.copy(out=ct[b][:], in_=pst[b][:]).then_inc(ctsem, 1)
                dst = out[4000 * s : 4000 * s + 4000].rearrange(
                    "(p t j) -> p t j", p=125, t=4
                )
                # the DMA trigger is async wrt the ACT engine pipe: gate on ctsem
                scalar.wait_ge(ctsem, g + 1)
                scalar.dma_start(
                    dst, ct[b][:].rearrange("p (t j) -> p t j", t=4)
                ).then_inc(osem[b], 16)
            scalar.wait_ge(psem_t, G + 1)
            nc.scalar.copy(out=ct_tail[:], in_=pstail[:]).then_inc(ctsem, 1)
            scalar.wait_ge(ctsem, G + 1)
            scalar.dma_start(out[STEP * KLOC : TLOC], ct_tail[:]).then_inc(osem0, 16)

    return nc


def audit_waits(nc, max_show=12):
    """Count on_wait entries per instruction; the TPB ISA allows ONE."""
    import json

    d = json.loads(nc.to_json_bytes())
    bad = []

    def walk(blocks):
        for bb in blocks:
            for i in bb.get("instructions", []):
                si = i.get("sync_info") or {}
                w = si.get("on_wait") or []
                if len(w) > 1:
                    bad.append(
                        (
                            i["name"],
                            i.get("opcode"),
                            len(w),
                            [s_.get("ant_name") for s_ in w],
                        )
                    )
            walk(bb.get("blocks", []))

    walk(d["functions"][0]["blocks"])
    return bad[:max_show], len(bad)


_NC_CACHE = {}


def _get_nc(reps=1):
    if reps not in _NC_CACHE:
        _NC_CACHE[reps] = build_nc(reps)
    return _NC_CACHE[reps]


def make_in_maps(mixture_w, est_mask, W):
    mixture_w = np.asarray(mixture_w, dtype=np.float32)
    est_mask = np.asarray(est_mask, dtype=np.float32)
    W = np.asarray(W, dtype=np.float32)
    wt = np.ascontiguousarray(W.T)                      # [N, L]
    ident = np.eye(8, dtype=np.float32)
    in_maps = []
    for c in range(8):
        b, h = c // 2, c % 2
        xx = np.stack(
            [
                mixture_w[b, :, h * KLOC : (h + 1) * KLOC],
                est_mask[b, :, h * KLOC : (h + 1) * KLOC],
            ]
        )
        in_maps.append({"x": np.ascontiguousarray(xx), "wt": wt, "ident": ident})
    return in_maps


def assemble(results):
    T = STEP * (K - 1) + L
    out = np.zeros((B, T), dtype=np.float32)
    for c in range(8):
        b, h = c // 2, c % 2
        out[b, h * STEP * KLOC : h * STEP * KLOC + TLOC] += results[c]["out"]
    return out


def run(mixture_w, est_mask, W, trace=False, reps=1, **spmd_kwargs):
    """Shard, run on 8 cores, gather. Returns (out, BassKernelResults)."""
    in_maps = make_in_maps(mixture_w, est_mask, W)
    nc = _get_nc(reps)
    kr = run_bass_kernel_spmd(
        nc, in_maps, core_ids=list(range(8)), trace=trace, **spmd_kwargs
    )
    return assemble(kr.results), kr


def kernel(mixture_w, est_mask, W):
    out, _ = run(mixture_w, est_mask, W)
    return out
